# revision 8
# baseline (speedup 1.0000x reference)
"""Trainium2 Bass kernel for 2-layer GAT (nn_GAT_59133109732231). v3: fp8.

Self-contained: kernel(**inputs) -> np.ndarray [100000, 2] float32.

Distribution (8 NeuronCores, SPMD), node-parallel:
  - nodes permuted: core c owns 120 superchunks (sc) x 128 rows (row 127 =
    trash row). sc 0..59 = half A, 60..119 = half B.
  - per sc, in-edges packed into 8 segments of 128 slots keyed by the
    SOURCE's (core-pair q, half h) octant -> one gather chunk each.
  - per layer: transform nodes with augmented weights [W | W@a_src | W@a_dst]
    (one matmul -> h + both attention dots), write 512B fp8 records
    [h0 |1| h1 |1| pad | asrc(bf16 pair) | pad]; TWO AllGathers (half A
    during half-B transform) -> record tables; per 12-sc group dma_gather
    512B records by int16 row id; per-edge weights w=exp(lrelu(asr+adst))
    via exp-max identity (scalar engine runs only Exp); weight the gathered
    records in place; aggregate + softmax denominator with fp8 DoubleRow
    one-hot matmuls in PSUM (host-streamed one-hot tables); adst per edge
    gathered via tiny oT matmuls. Layer-2 transform / post-MLP (Wp1@Wp2
    folded) chained in SBUF per 2-sc pair.
"""
import os
import sys

import numpy as np
import ml_dtypes

for _p in ("/opt/trn_rl_repo", "/root/.axon_site/_ro/trn_rl_repo"):
    if os.path.isdir(_p) and _p not in sys.path:
        sys.path.append(_p)

N = 100000
NCORES = 8
S_SC = 120            # superchunks per core
S_H = 60              # sc per half
ROWS_CORE = S_SC * 128
ROWS_H = S_H * 128    # 7680
REC = 512             # fp8 cols per record (512 B)
GRP = 12              # sc per E-group
NGRP = S_SC // GRP    # 10
NEG_SLOPE = 0.2
NSEG = 8              # segments (octants) per sc

bf16 = ml_dtypes.bfloat16
fp8 = ml_dtypes.float8_e4m3


# ----------------------------------------------------------------- host prep
def build_plan(edge_index):
    edge_index = np.asarray(edge_index)
    src = edge_index[0].astype(np.int64)
    dst = edge_index[1].astype(np.int64)

    deg = np.bincount(dst, minlength=N)
    order = np.argsort(-deg, kind="stable")
    owner = np.empty(N, dtype=np.int32)
    snake = np.tile(np.concatenate([np.arange(8), np.arange(7, -1, -1)]),
                    N // 16 + 1)[:N]
    owner[order] = snake.astype(np.int32)

    # half assignment: alternate within each core by degree rank
    half = np.empty(N, dtype=np.int32)
    for c in range(NCORES):
        nodes = order[owner[order] == c]
        half[nodes] = np.arange(len(nodes)) % 2

    # octant of an edge = (src core-pair, src half)
    e_oct = (owner[src] // 2) * 2 + half[src]
    qd = np.zeros((N, NSEG), dtype=np.int32)
    np.add.at(qd, (dst, e_oct), 1)

    sc_of = np.empty(N, dtype=np.int32)
    row_of = np.empty(N, dtype=np.int32)
    for c in range(NCORES):
        for h in range(2):
            nodes = np.where((owner == c) & (half == h))[0]
            nodes = nodes[np.argsort(-deg[nodes], kind="stable")]
            loads = np.zeros((S_H, NSEG), dtype=np.int32)
            counts = np.zeros(S_H, dtype=np.int32)
            tot = np.zeros(S_H, dtype=np.int32)
            big = 1.0e9
            for n in nodes:
                after = loads + qd[n][None, :]
                ok = (after <= 128).all(axis=1) & (counts < 127)
                key = after.max(axis=1).astype(np.float64) + tot * 1e-6 + (~ok) * big
                k = int(np.argmin(key))
                assert ok[k], "packing failed"
                sc_of[n] = h * S_H + k
                row_of[n] = counts[k]
                counts[k] += 1
                loads[k] += qd[n]
                tot[k] += deg[n]
    # eidx value: row within the (q, h) table region
    rowq_of = ((owner % 2) * ROWS_H + (sc_of % S_H) * 128 + row_of).astype(np.int32)

    e_core = owner[dst]
    e_sc = sc_of[dst]
    e_rowq = rowq_of[src]
    e_dloc = row_of[dst]

    plans = []
    for c in range(NCORES):
        eidx = np.zeros((S_SC, NSEG, 128), dtype=np.int16)
        dloc = np.full((S_SC, NSEG, 128), 127, dtype=np.int32)
        m = e_core == c
        sc_c, o_c, rq_c, dl_c = e_sc[m], e_oct[m], e_rowq[m], e_dloc[m]
        o = np.lexsort((o_c, sc_c))
        sc_c, o_c, rq_c, dl_c = sc_c[o], o_c[o], rq_c[o], dl_c[o]
        key = sc_c * NSEG + o_c
        pos = np.arange(len(key)) - np.searchsorted(key, key, side="left")
        assert pos.max() < 128
        eidx[sc_c, o_c, pos] = rq_c.astype(np.int16)
        dloc[sc_c, o_c, pos] = dl_c
        plans.append(dict(eidx=eidx, dloc=dloc))
    return dict(owner=owner, sc_of=sc_of, row_of=row_of, plans=plans)


def make_core_inputs(plan, inputs):
    x = np.asarray(inputs["x"], dtype=np.float32)

    def amat(a):
        a = np.asarray(a, dtype=np.float32)
        m = np.zeros((256, 2), dtype=np.float32)
        m[0:128, 0] = a[0]
        m[128:256, 1] = a[1]
        return m

    W1 = np.asarray(inputs["W1"], dtype=np.float32)
    W2 = np.asarray(inputs["W2"], dtype=np.float32)
    W1aug = np.concatenate(
        [W1, W1 @ amat(inputs["a_src1"]), W1 @ amat(inputs["a_dst1"])], axis=1)
    W2aug = np.concatenate(
        [W2, W2 @ amat(inputs["a_src2"]), W2 @ amat(inputs["a_dst2"])], axis=1)
    Wp1 = np.asarray(inputs["Wp1"], dtype=np.float32)
    Wp2 = np.asarray(inputs["Wp2"], dtype=np.float32)
    Wpf = Wp1 @ Wp2                                   # [256, 2]
    bpf = np.asarray(inputs["bp1"], np.float32) @ Wp2 + np.asarray(
        inputs["bp2"], np.float32)                    # [2]

    rep = lambda v, d: np.broadcast_to(
        np.asarray(v, dtype=np.float32)[None, :], (128, d)).copy()
    shared = dict(
        w1aug=W1aug.astype(bf16),
        w2aug=W2aug.astype(bf16),
        wpf=Wpf.astype(bf16),
        b1=rep(inputs["b1"], 256).reshape(128, 2, 128).copy(),
        b2=rep(inputs["b2"], 256).reshape(128, 2, 128).copy(),
        bpf=rep(bpf, 2),
        identf8=np.eye(128, dtype=np.float32).astype(fp8),
        identbf=np.eye(128, dtype=np.float32).astype(bf16),
    )

    owner, sc_of, row_of = plan["owner"], plan["sc_of"], plan["row_of"]
    ar = np.arange(128)
    cores = []
    for c in range(NCORES):
        xpT = np.zeros((128, ROWS_CORE), dtype=np.float32)
        nodes = np.where(owner == c)[0]
        xpT[:, sc_of[nodes] * 128 + row_of[nodes]] = x[nodes].T
        p = plan["plans"][c]
        # reorder octants hu-major: qh = hu*4 + q  (oct was q*2+hu)
        perm = [0, 2, 4, 6, 1, 3, 5, 7]
        dlocp = p["dloc"][:, perm, :]
        eidxp = p["eidx"][:, perm, :]
        # one-hot streams: chunk index c = qh*GRP + k within group g
        A = (dlocp[:, :, :, None] == ar).astype(np.uint8)  # [sc, qh, s, m]
        A5 = A.reshape(NGRP, GRP, NSEG, 128, 128)
        ohs = np.ascontiguousarray(
            A5.transpose(3, 0, 2, 1, 4).reshape(128, NGRP * NSEG * GRP * 128))
        ots = np.ascontiguousarray(
            A5.transpose(4, 0, 2, 1, 3).reshape(128, NGRP * NSEG * GRP * 128))
        # eidx wrap: per (g, qh) gather of GRP*128 idxs
        ei = eidxp.reshape(NGRP, GRP, NSEG, 128)
        eiw = np.zeros((128, NGRP * NSEG * GRP * 8), dtype=np.int16)
        C = GRP * 8  # cols per gather = GRP*128/16
        for g in range(NGRP):
            for qh in range(NSEG):
                lst = ei[g, :, qh, :].reshape(-1)          # [GRP*128]
                w = lst.reshape(GRP * 8, 16).T             # [16, GRP*8]
                eiw[:, (g * NSEG + qh) * C:(g * NSEG + qh + 1) * C] = np.tile(w, (8, 1))
        cores.append(dict(xpT=xpT.astype(bf16),
                          eidx=eiw,
                          ohs=ohs.astype(fp8),
                          ots=ots.astype(fp8)))
    return cores, shared


# -------------------------------------------------------------- bass program
def build_nc():
    import concourse.bass as bass
    import concourse.bacc as bacc
    import concourse.mybir as mybir
    import concourse.tile as tile

    F32, BF, F8, I16 = (mybir.dt.float32, mybir.dt.bfloat16,
                        mybir.dt.float8e4, mybir.dt.int16)
    AF = mybir.ActivationFunctionType
    ALU = mybir.AluOpType
    PM = mybir.MatmulPerfMode

    nc = bacc.Bacc("TRN2", target_bir_lowering=False, debug=False,
                   num_devices=8, num_swdge_queues=4)

    din = {}
    for name, shape, dt in [
        ("xpT", [128, ROWS_CORE], BF),
        ("eidx", [128, NGRP * NSEG * GRP * 8], I16),
        ("ohs", [128, NGRP * NSEG * GRP * 128], F8),
        ("ots", [128, NGRP * NSEG * GRP * 128], F8),
        ("w1aug", [128, 260], BF),
        ("w2aug", [256, 260], BF),
        ("wpf", [256, 2], BF),
        ("b1", [128, 2, 128], F32), ("b2", [128, 2, 128], F32),
        ("bpf", [128, 2], F32),
        ("identf8", [128, 128], F8),
        ("identbf", [128, 128], BF),
    ]:
        din[name] = nc.dram_tensor(name, shape, dt, kind="ExternalInput")
    y_d = nc.dram_tensor("y", [ROWS_CORE, 2], F32, kind="ExternalOutput")
    sh = {}
    tab = {}
    for L in range(2):
        for h in range(2):
            sh[L, h] = nc.dram_tensor(f"sh{L}{h}", [ROWS_H, REC], F8,
                                      kind="Internal")
            tab[L, h] = nc.dram_tensor(f"tab{L}{h}", [8 * ROWS_H, REC], F8,
                                       kind="Internal", addr_space="Shared")

    with tile.TileContext(nc) as tc:
        import contextlib
        ctx = contextlib.ExitStack()
        with ctx:
            pp = ctx.enter_context(tc.tile_pool(name="pp", bufs=1))
            sbT = ctx.enter_context(tc.tile_pool(name="sbT", bufs=2))
            gp = ctx.enter_context(tc.tile_pool(name="gp", bufs=2))
            go = ctx.enter_context(tc.tile_pool(name="go", bufs=2))
            sbE = ctx.enter_context(tc.tile_pool(name="sbE", bufs=2))
            psA = ctx.enter_context(tc.tile_pool(name="psA", bufs=2, space="PSUM"))
            psB = ctx.enter_context(tc.tile_pool(name="psB", bufs=2, space="PSUM"))
            psC = ctx.enter_context(tc.tile_pool(name="psC", bufs=2, space="PSUM"))

            # persistent SBUF
            P = {}
            for name in ("eidx", "w1aug", "b1", "b2", "bpf", "identf8", "identbf"):
                t = pp.tile(list(din[name].shape), din[name].dtype, tag=f"p_{name}")
                nc.sync.dma_start(t[:], din[name].ap())
                P[name] = t
            w2s = pp.tile([128, 2, 260], BF, tag="p_w2")
            nc.sync.dma_start(w2s[:, 0, :], din["w2aug"].ap()[0:128, :])
            nc.sync.dma_start(w2s[:, 1, :], din["w2aug"].ap()[128:256, :])
            wpf = pp.tile([128, 2, 2], BF, tag="p_wpf")
            nc.sync.dma_start(wpf[:, 0, :], din["wpf"].ap()[0:128, :])
            nc.sync.dma_start(wpf[:, 1, :], din["wpf"].ap()[128:256, :])
            aab = pp.tile([128, S_SC, 4], BF, tag="p_aab")

            shp = {k: v.ap().rearrange("(s p) c -> p s c", p=128)
                   for k, v in sh.items()}

            def rec_build(ph, L, sc0):
                """ph [128,2,512] f32 psum (cols 0:260 = h|asrc|adst) ->
                fp8 records + aab; DMA to shard."""
                h = sc0 // S_H
                lsc = sc0 % S_H
                rec = sbT.tile([128, 2, REC], F8, tag="rec")
                nc.scalar.activation(rec[:, :, 0:128], ph[:, :, 0:128], AF.Copy)
                nc.vector.tensor_copy(rec[:, :, 129:257], ph[:, :, 128:256])
                nc.vector.memset(rec[:, :, 128:129], 1.0)
                nc.vector.memset(rec[:, :, 257:258], 1.0)
                nc.scalar.activation(rec[:, :, 260:264].bitcast(BF),
                                     ph[:, :, 256:258], AF.Copy)
                nc.vector.tensor_copy(aab[:, sc0:sc0 + 2, :], ph[:, :, 256:260])
                nc.sync.dma_start(shp[L, h][:, lsc:lsc + 2, :], rec[:])

            def allgather(L, h):
                import concourse.mybir as mybir
                nc.gpsimd.collective_compute(
                    "AllGather", mybir.AluOpType.bypass,
                    replica_groups=[list(range(8))],
                    ins=[sh[L, h].ap()], outs=[tab[L, h].ap()])

            # ---------------- phase T1
            for g in range(S_H):   # 60 groups of 2 sc
                sc0 = g * 2
                xt = sbT.tile([128, 256], BF, tag="xt1")
                nc.sync.dma_start(xt[:], din["xpT"].ap()[:, sc0 * 128:(sc0 + 2) * 128])
                ph = psA.tile([128, 2, 512], F32, tag="po")
                for ki in range(2):
                    nc.tensor.matmul(ph[:, ki, 0:260], lhsT=xt[:, ki * 128:(ki + 1) * 128],
                                     rhs=P["w1aug"][:], start=True, stop=True)
                rec_build(ph, 0, sc0)
                if g == S_H // 2 - 1:
                    allgather(0, 0)
            allgather(0, 1)

            # ---------------- E phase (layer L); chains T2 (L=0) or post (L=1)
            def e_group(L, g):
                half = g // (NGRP // 2)
                lsc0 = (g * GRP) % S_H
                CH = NSEG * GRP          # 96 chunks
                gtA = gp.tile([128, 4, GRP, REC], F8, tag="gtA")
                gtB = gp.tile([128, 4, GRP, REC], F8, tag="gtB")
                srec = gp.tile([128, GRP, REC], F8, tag="srec")
                nc.sync.dma_start(srec[:], shp[L, half][:, lsc0:lsc0 + GRP, :])
                oh = go.tile([128, NSEG, GRP, 128], F8, tag="oh")
                nc.sync.dma_start(oh[:], din["ohs"].ap()[:, g * CH * 128:(g + 1) * CH * 128])
                ot = go.tile([128, NSEG, GRP, 128], F8, tag="ot")
                nc.sync.dma_start(ot[:], din["ots"].ap()[:, g * CH * 128:(g + 1) * CH * 128])
                C = GRP * 8
                for qh in range(NSEG):
                    hu = qh // 4
                    q = qh % 4
                    tgt = gtA if hu == 0 else gtB
                    nc.gpsimd.dma_gather(
                        tgt[:, q, :, :],
                        tab[L, hu].ap()[q * 2 * ROWS_H:(q + 1) * 2 * ROWS_H, :],
                        P["eidx"][:, (g * NSEG + qh) * C:(g * NSEG + qh + 1) * C],
                        GRP * 128, GRP * 128, REC,
                        single_packet=False, queue_num=qh % 4)
                # per-slot adst via oT matmuls
                pae = psB.tile([128, CH, 2], F32, tag="aux")
                for k in range(GRP):
                    for qh in range(NSEG):
                        cidx = qh * GRP + k
                        nc.tensor.matmul(pae[:, cidx, :],
                                         lhsT=ot[:, qh, k, :],
                                         rhs=aab[:, g * GRP + k, 2:4],
                                         start=True, stop=True)
                # logits -> weights; layout [A(48) | self(12) | B(48)]
                HC = CH // 2
                lg = sbE.tile([128, CH + GRP, 2], F32, tag="lg")
                gtfA = gtA[:].rearrange("p a b c -> p (a b) c")
                gtfB = gtB[:].rearrange("p a b c -> p (a b) c")
                nc.vector.tensor_tensor(lg[:, 0:HC, :],
                                        gtfA[:, :, 260:264].bitcast(BF),
                                        pae[:, 0:HC, :], ALU.add)
                nc.vector.tensor_tensor(lg[:, HC:HC + GRP, :],
                                        aab[:, g * GRP:(g + 1) * GRP, 0:2],
                                        aab[:, g * GRP:(g + 1) * GRP, 2:4], ALU.add)
                nc.vector.tensor_tensor(lg[:, HC + GRP:, :],
                                        gtfB[:, :, 260:264].bitcast(BF),
                                        pae[:, HC:CH, :], ALU.add)
                e1 = sbE.tile([128, CH + GRP, 2], BF, tag="e1")
                e2 = sbE.tile([128, CH + GRP, 2], BF, tag="e2")
                we = sbE.tile([128, CH + GRP, 2], BF, tag="we")
                NA = HC + GRP
                for lo, hi in ((0, NA), (NA, CH + GRP)):
                    nc.scalar.activation(e1[:, lo:hi, :], lg[:, lo:hi, :], AF.Exp)
                    nc.scalar.activation(e2[:, lo:hi, :], lg[:, lo:hi, :],
                                         AF.Exp, scale=NEG_SLOPE)
                    nc.vector.tensor_tensor(we[:, lo:hi, :], e1[:, lo:hi, :],
                                            e2[:, lo:hi, :], ALU.max)
                # weight records in place (per head block of 129 cols)
                for hd in range(2):
                    nc.vector.tensor_tensor(
                        gtfA[:, :, hd * 129:hd * 129 + 129],
                        gtfA[:, :, hd * 129:hd * 129 + 129],
                        we[:, 0:HC, hd:hd + 1].to_broadcast([128, HC, 129]),
                        ALU.mult)
                    nc.vector.tensor_tensor(
                        srec[:, :, hd * 129:hd * 129 + 129],
                        srec[:, :, hd * 129:hd * 129 + 129],
                        we[:, HC:HC + GRP, hd:hd + 1].to_broadcast([128, GRP, 129]),
                        ALU.mult)
                    nc.vector.tensor_tensor(
                        gtfB[:, :, hd * 129:hd * 129 + 129],
                        gtfB[:, :, hd * 129:hd * 129 + 129],
                        we[:, HC + GRP:, hd:hd + 1].to_broadcast([128, HC, 129]),
                        ALU.mult)
                # aggregate + normalize per 2-sc pair
                for pr in range(GRP // 2):
                    po = psA.tile([128, 2, 512], F32, tag="po")
                    for ki in range(2):
                        k = pr * 2 + ki
                        nc.tensor.matmul(po[:, ki, 0:258], lhsT=P["identf8"][:],
                                         rhs=srec[:, k, 0:258], start=True, stop=False)
                        for qp in range(NSEG // 2):
                            gtX = gtA if qp < 2 else gtB
                            ql = (qp % 2) * 2
                            nc.tensor.matmul(
                                po[:, ki, 0:258],
                                lhsT=oh[:, 2 * qp:2 * qp + 2, k, :],
                                rhs=gtX[:, ql:ql + 2, k, 0:258],
                                start=False, stop=(qp == 3),
                                perf_mode=PM.DoubleRow)
                    po4 = po[:, :, 0:258].rearrange("p s (h c) -> p s h c", h=2)
                    den = sbE.tile([128, 2, 2, 1], F32, tag="den")
                    nc.vector.tensor_copy(den[:], po4[:, :, :, 128:129])
                    nc.vector.tensor_scalar_add(den[:], den[:], 1e-16)
                    nc.vector.reciprocal(den[:], den[:])
                    of = sbE.tile([128, 2, 2, 128], F32, tag="of")
                    nc.vector.tensor_tensor(
                        of[:], po4[:, :, :, 0:128],
                        den[:].to_broadcast([128, 2, 2, 128]),
                        ALU.mult)
                    bias = P["b1"] if L == 0 else P["b2"]
                    nc.vector.tensor_tensor(
                        of[:], of[:],
                        bias[:].unsqueeze(1).to_broadcast([128, 2, 2, 128]),
                        ALU.add)
                    o2 = sbE.tile([128, 2, 2, 128], BF, tag="o2")
                    nc.scalar.activation(o2[:], of[:], AF.Relu)
                    sc0 = g * GRP + pr * 2
                    if L == 0:
                        # chained T2: transpose o2, transform, write layer-2 recs
                        hT = sbT.tile([128, 2, 2, 128], BF, tag="hT")
                        for ki in range(2):
                            for ch in range(2):
                                pt = psC.tile([128, 128], BF, tag="pt")
                                nc.tensor.transpose(pt[:], o2[:, ki, ch, :],
                                                    P["identbf"][:])
                                nc.scalar.activation(hT[:, ki, ch, :], pt[:], AF.Copy)
                        ph = psA.tile([128, 2, 512], F32, tag="po")
                        for ki in range(2):
                            for ch in range(2):
                                nc.tensor.matmul(ph[:, ki, 0:260],
                                                 lhsT=hT[:, ki, ch, :],
                                                 rhs=w2s[:, ch, :],
                                                 start=(ch == 0), stop=(ch == 1))
                        rec_build(ph, 1, sc0)
                    else:
                        # chained post-MLP: y = sigmoid(o2 @ Wpf + bpf)
                        oT = sbT.tile([128, 2, 2, 128], BF, tag="hT")
                        for ki in range(2):
                            for ch in range(2):
                                pt = psC.tile([128, 128], BF, tag="pt")
                                nc.tensor.transpose(pt[:], o2[:, ki, ch, :],
                                                    P["identbf"][:])
                                nc.scalar.activation(oT[:, ki, ch, :], pt[:], AF.Copy)
                        yp = psC.tile([128, 2, 2], F32, tag="pt")
                        for ki in range(2):
                            for ch in range(2):
                                nc.tensor.matmul(yp[:, ki, :],
                                                 lhsT=oT[:, ki, ch, :],
                                                 rhs=wpf[:, ch, :],
                                                 start=(ch == 0), stop=(ch == 1))
                        yv = sbE.tile([128, 2, 2], F32, tag="yv")
                        nc.vector.tensor_tensor(
                            yv[:], yp[:],
                            P["bpf"][:].unsqueeze(1).to_broadcast([128, 2, 2]),
                            ALU.add)
                        sg = sbE.tile([128, 2, 2], F32, tag="sg")
                        nc.scalar.activation(sg[:], yv[:], AF.Exp, scale=-1.0)
                        nc.vector.tensor_scalar_add(sg[:], sg[:], 1.0)
                        nc.vector.reciprocal(sg[:], sg[:])
                        ypd = y_d.ap().rearrange("(s p) c -> p s c", p=128)
                        nc.scalar.dma_start(ypd[:, sc0:sc0 + 2, :], sg[:])

            for g in range(NGRP):
                e_group(0, g)
                if g == NGRP // 2 - 1:
                    allgather(1, 0)
            allgather(1, 1)
            for g in range(NGRP):
                e_group(1, g)
    nc.compile()
    return nc


_NC_CACHE = None


def kernel(**inputs):
    global _NC_CACHE
    from concourse.bass_utils import run_bass_kernel_spmd

    plan = build_plan(inputs["edge_index"])
    cores, shared = make_core_inputs(plan, inputs)

    if _NC_CACHE is None:
        _NC_CACHE = build_nc()
    nc = _NC_CACHE

    in_maps = []
    for c in range(8):
        m = dict(shared)
        m.update(cores[c])
        in_maps.append({k: np.ascontiguousarray(v) for k, v in m.items()})

    res = run_bass_kernel_spmd(nc, in_maps, core_ids=list(range(8)))

    owner, sc_of, row_of = plan["owner"], plan["sc_of"], plan["row_of"]
    y = np.zeros((N, 2), dtype=np.float32)
    for c in range(8):
        yc = res.results[c]["y"]
        nodes = np.where(owner == c)[0]
        y[nodes] = yc[sc_of[nodes] * 128 + row_of[nodes]]
    return y


# revision 10
# speedup vs baseline: 1.0074x; 1.0074x over previous
"""Trainium2 Bass kernel for 2-layer GAT (nn_GAT_59133109732231). v3: fp8.

Self-contained: kernel(**inputs) -> np.ndarray [100000, 2] float32.

Distribution (8 NeuronCores, SPMD), node-parallel:
  - nodes permuted: core c owns 120 superchunks (sc) x 128 rows (row 127 =
    trash row). sc 0..59 = half A, 60..119 = half B.
  - per sc, in-edges packed into 8 segments of 128 slots keyed by the
    SOURCE's (core-pair q, half h) octant -> one gather chunk each.
  - per layer: transform nodes with augmented weights [W | W@a_src | W@a_dst]
    (one matmul -> h + both attention dots), write 512B fp8 records
    [h0 |1| h1 |1| pad | asrc(bf16 pair) | pad]; TWO AllGathers (half A
    during half-B transform) -> record tables; per 12-sc group dma_gather
    512B records by int16 row id; per-edge weights w=exp(lrelu(asr+adst))
    via exp-max identity (scalar engine runs only Exp); weight the gathered
    records in place; aggregate + softmax denominator with fp8 DoubleRow
    one-hot matmuls in PSUM (host-streamed one-hot tables); adst per edge
    gathered via tiny oT matmuls. Layer-2 transform / post-MLP (Wp1@Wp2
    folded) chained in SBUF per 2-sc pair.
"""
import os
import sys

import numpy as np
import ml_dtypes

for _p in ("/opt/trn_rl_repo", "/root/.axon_site/_ro/trn_rl_repo"):
    if os.path.isdir(_p) and _p not in sys.path:
        sys.path.append(_p)

N = 100000
NCORES = 8
S_SC = 120            # superchunks per core
S_H = 60              # sc per half
ROWS_CORE = S_SC * 128
ROWS_H = S_H * 128    # 7680
REC = 512             # fp8 cols per record (512 B)
GRP = 12              # sc per E-group
NGRP = S_SC // GRP    # 10
NEG_SLOPE = 0.2
NSEG = 8              # segments (octants) per sc

bf16 = ml_dtypes.bfloat16
fp8 = ml_dtypes.float8_e4m3


# ----------------------------------------------------------------- host prep
def build_plan(edge_index):
    edge_index = np.asarray(edge_index)
    src = edge_index[0].astype(np.int64)
    dst = edge_index[1].astype(np.int64)

    deg = np.bincount(dst, minlength=N)
    order = np.argsort(-deg, kind="stable")
    owner = np.empty(N, dtype=np.int32)
    snake = np.tile(np.concatenate([np.arange(8), np.arange(7, -1, -1)]),
                    N // 16 + 1)[:N]
    owner[order] = snake.astype(np.int32)

    # half assignment: alternate within each core by degree rank
    half = np.empty(N, dtype=np.int32)
    for c in range(NCORES):
        nodes = order[owner[order] == c]
        half[nodes] = np.arange(len(nodes)) % 2

    # octant of an edge = (src core-pair, src half)
    e_oct = (owner[src] // 2) * 2 + half[src]
    qd = np.zeros((N, NSEG), dtype=np.int32)
    np.add.at(qd, (dst, e_oct), 1)

    sc_of = np.empty(N, dtype=np.int32)
    row_of = np.empty(N, dtype=np.int32)
    for c in range(NCORES):
        for h in range(2):
            nodes = np.where((owner == c) & (half == h))[0]
            nodes = nodes[np.argsort(-deg[nodes], kind="stable")]
            loads = np.zeros((S_H, NSEG), dtype=np.int32)
            counts = np.zeros(S_H, dtype=np.int32)
            tot = np.zeros(S_H, dtype=np.int32)
            big = 1.0e9
            for n in nodes:
                after = loads + qd[n][None, :]
                ok = (after <= 128).all(axis=1) & (counts < 127)
                key = after.max(axis=1).astype(np.float64) + tot * 1e-6 + (~ok) * big
                k = int(np.argmin(key))
                assert ok[k], "packing failed"
                sc_of[n] = h * S_H + k
                row_of[n] = counts[k]
                counts[k] += 1
                loads[k] += qd[n]
                tot[k] += deg[n]
    # eidx value: row within the (q, h) table region
    rowq_of = ((owner % 2) * ROWS_H + (sc_of % S_H) * 128 + row_of).astype(np.int32)

    e_core = owner[dst]
    e_sc = sc_of[dst]
    e_rowq = rowq_of[src]
    e_dloc = row_of[dst]

    plans = []
    for c in range(NCORES):
        eidx = np.zeros((S_SC, NSEG, 128), dtype=np.int16)
        dloc = np.full((S_SC, NSEG, 128), 127, dtype=np.int32)
        m = e_core == c
        sc_c, o_c, rq_c, dl_c = e_sc[m], e_oct[m], e_rowq[m], e_dloc[m]
        o = np.lexsort((o_c, sc_c))
        sc_c, o_c, rq_c, dl_c = sc_c[o], o_c[o], rq_c[o], dl_c[o]
        key = sc_c * NSEG + o_c
        pos = np.arange(len(key)) - np.searchsorted(key, key, side="left")
        assert pos.max() < 128
        eidx[sc_c, o_c, pos] = rq_c.astype(np.int16)
        dloc[sc_c, o_c, pos] = dl_c
        plans.append(dict(eidx=eidx, dloc=dloc))
    return dict(owner=owner, sc_of=sc_of, row_of=row_of, plans=plans)


def make_core_inputs(plan, inputs):
    x = np.asarray(inputs["x"], dtype=np.float32)

    def amat(a):
        a = np.asarray(a, dtype=np.float32)
        m = np.zeros((256, 2), dtype=np.float32)
        m[0:128, 0] = a[0]
        m[128:256, 1] = a[1]
        return m

    W1 = np.asarray(inputs["W1"], dtype=np.float32)
    W2 = np.asarray(inputs["W2"], dtype=np.float32)
    W1aug = np.concatenate(
        [W1, W1 @ amat(inputs["a_src1"]), W1 @ amat(inputs["a_dst1"])], axis=1)
    W2aug = np.concatenate(
        [W2, W2 @ amat(inputs["a_src2"]), W2 @ amat(inputs["a_dst2"])], axis=1)
    Wp1 = np.asarray(inputs["Wp1"], dtype=np.float32)
    Wp2 = np.asarray(inputs["Wp2"], dtype=np.float32)
    Wpf = Wp1 @ Wp2                                   # [256, 2]
    bpf = np.asarray(inputs["bp1"], np.float32) @ Wp2 + np.asarray(
        inputs["bp2"], np.float32)                    # [2]

    rep = lambda v, d: np.broadcast_to(
        np.asarray(v, dtype=np.float32)[None, :], (128, d)).copy()
    shared = dict(
        w1aug=W1aug.astype(bf16),
        w2aug=W2aug.astype(bf16),
        wpf=Wpf.astype(bf16),
        b1=rep(inputs["b1"], 256).reshape(128, 2, 128).copy(),
        b2=rep(inputs["b2"], 256).reshape(128, 2, 128).copy(),
        bpf=rep(bpf, 2),
        identf8=np.eye(128, dtype=np.float32).astype(fp8),
        identbf=np.eye(128, dtype=np.float32).astype(bf16),
    )

    owner, sc_of, row_of = plan["owner"], plan["sc_of"], plan["row_of"]
    ar = np.arange(128)
    cores = []
    for c in range(NCORES):
        xpT = np.zeros((128, ROWS_CORE), dtype=np.float32)
        nodes = np.where(owner == c)[0]
        xpT[:, sc_of[nodes] * 128 + row_of[nodes]] = x[nodes].T
        p = plan["plans"][c]
        # reorder octants hu-major: qh = hu*4 + q  (oct was q*2+hu)
        perm = [0, 2, 4, 6, 1, 3, 5, 7]
        dlocp = p["dloc"][:, perm, :]
        eidxp = p["eidx"][:, perm, :]
        # one-hot streams: chunk index c = qh*GRP + k within group g
        A = (dlocp[:, :, :, None] == ar).astype(np.uint8)  # [sc, qh, s, m]
        A5 = A.reshape(NGRP, GRP, NSEG, 128, 128)
        ohs = np.ascontiguousarray(
            A5.transpose(3, 0, 2, 1, 4).reshape(128, NGRP * NSEG * GRP * 128))
        ots = np.ascontiguousarray(
            A5.transpose(4, 0, 2, 1, 3).reshape(128, NGRP * NSEG * GRP * 128))
        # eidx wrap: per (g, qh) gather of GRP*128 idxs
        ei = eidxp.reshape(NGRP, GRP, NSEG, 128)
        eiw = np.zeros((128, NGRP * NSEG * GRP * 8), dtype=np.int16)
        C = GRP * 8  # cols per gather = GRP*128/16
        for g in range(NGRP):
            for qh in range(NSEG):
                lst = ei[g, :, qh, :].reshape(-1)          # [GRP*128]
                w = lst.reshape(GRP * 8, 16).T             # [16, GRP*8]
                eiw[:, (g * NSEG + qh) * C:(g * NSEG + qh + 1) * C] = np.tile(w, (8, 1))
        cores.append(dict(xpT=xpT.astype(bf16),
                          eidx=eiw,
                          ohs=ohs.astype(fp8),
                          ots=ots.astype(fp8)))
    return cores, shared


# -------------------------------------------------------------- bass program
def build_nc():
    import concourse.bass as bass
    import concourse.bacc as bacc
    import concourse.mybir as mybir
    import concourse.tile as tile

    F32, BF, F8, I16 = (mybir.dt.float32, mybir.dt.bfloat16,
                        mybir.dt.float8e4, mybir.dt.int16)
    AF = mybir.ActivationFunctionType
    ALU = mybir.AluOpType
    PM = mybir.MatmulPerfMode

    nc = bacc.Bacc("TRN2", target_bir_lowering=False, debug=False,
                   num_devices=8, num_swdge_queues=4)

    din = {}
    for name, shape, dt in [
        ("xpT", [128, ROWS_CORE], BF),
        ("eidx", [128, NGRP * NSEG * GRP * 8], I16),
        ("ohs", [128, NGRP * NSEG * GRP * 128], F8),
        ("ots", [128, NGRP * NSEG * GRP * 128], F8),
        ("w1aug", [128, 260], BF),
        ("w2aug", [256, 260], BF),
        ("wpf", [256, 2], BF),
        ("b1", [128, 2, 128], F32), ("b2", [128, 2, 128], F32),
        ("bpf", [128, 2], F32),
        ("identf8", [128, 128], F8),
        ("identbf", [128, 128], BF),
    ]:
        din[name] = nc.dram_tensor(name, shape, dt, kind="ExternalInput")
    y_d = nc.dram_tensor("y", [ROWS_CORE, 2], F32, kind="ExternalOutput")
    sh = {}
    tab = {}
    for L in range(2):
        for h in range(2):
            sh[L, h] = nc.dram_tensor(f"sh{L}{h}", [ROWS_H, REC], F8,
                                      kind="Internal")
            tab[L, h] = nc.dram_tensor(f"tab{L}{h}", [8 * ROWS_H, REC], F8,
                                       kind="Internal", addr_space="Shared")

    with tile.TileContext(nc) as tc:
        import contextlib
        ctx = contextlib.ExitStack()
        with ctx:
            pp = ctx.enter_context(tc.tile_pool(name="pp", bufs=1))
            sbT = ctx.enter_context(tc.tile_pool(name="sbT", bufs=2))
            gp = ctx.enter_context(tc.tile_pool(name="gp", bufs=2))
            go = ctx.enter_context(tc.tile_pool(name="go", bufs=2))
            sbE = ctx.enter_context(tc.tile_pool(name="sbE", bufs=2))
            psA = ctx.enter_context(tc.tile_pool(name="psA", bufs=2, space="PSUM"))
            psB = ctx.enter_context(tc.tile_pool(name="psB", bufs=2, space="PSUM"))
            psC = ctx.enter_context(tc.tile_pool(name="psC", bufs=2, space="PSUM"))

            # persistent SBUF
            P = {}
            for name in ("eidx", "w1aug", "b1", "b2", "bpf", "identf8", "identbf"):
                t = pp.tile(list(din[name].shape), din[name].dtype, tag=f"p_{name}")
                nc.sync.dma_start(t[:], din[name].ap())
                P[name] = t
            w2s = pp.tile([128, 2, 260], BF, tag="p_w2")
            nc.sync.dma_start(w2s[:, 0, :], din["w2aug"].ap()[0:128, :])
            nc.sync.dma_start(w2s[:, 1, :], din["w2aug"].ap()[128:256, :])
            wpf = pp.tile([128, 2, 2], BF, tag="p_wpf")
            nc.sync.dma_start(wpf[:, 0, :], din["wpf"].ap()[0:128, :])
            nc.sync.dma_start(wpf[:, 1, :], din["wpf"].ap()[128:256, :])
            aab = pp.tile([128, S_SC, 4], BF, tag="p_aab")

            shp = {k: v.ap().rearrange("(s p) c -> p s c", p=128)
                   for k, v in sh.items()}

            def rec_build(ph, L, sc0):
                """ph [128,2,512] f32 psum (cols 0:260 = h|asrc|adst) ->
                fp8 records + aab; DMA to shard."""
                h = sc0 // S_H
                lsc = sc0 % S_H
                rec = sbT.tile([128, 2, REC], F8, tag="rec")
                nc.scalar.activation(rec[:, :, 0:128], ph[:, :, 0:128], AF.Copy)
                nc.vector.tensor_copy(rec[:, :, 129:257], ph[:, :, 128:256])
                nc.vector.memset(rec[:, :, 128:129], 1.0)
                nc.vector.memset(rec[:, :, 257:258], 1.0)
                nc.scalar.activation(rec[:, :, 260:264].bitcast(BF),
                                     ph[:, :, 256:258], AF.Copy)
                nc.vector.tensor_copy(aab[:, sc0:sc0 + 2, :], ph[:, :, 256:260])
                nc.sync.dma_start(shp[L, h][:, lsc:lsc + 2, :], rec[:])

            def allgather(L, h):
                import concourse.mybir as mybir
                nc.gpsimd.collective_compute(
                    "AllGather", mybir.AluOpType.bypass,
                    replica_groups=[list(range(8))],
                    ins=[sh[L, h].ap()], outs=[tab[L, h].ap()])

            # ---------------- phase T1
            for g in range(S_H):   # 60 groups of 2 sc
                sc0 = g * 2
                xt = sbT.tile([128, 256], BF, tag="xt1")
                nc.sync.dma_start(xt[:], din["xpT"].ap()[:, sc0 * 128:(sc0 + 2) * 128])
                ph = psA.tile([128, 2, 512], F32, tag="po")
                for ki in range(2):
                    nc.tensor.matmul(ph[:, ki, 0:260], lhsT=xt[:, ki * 128:(ki + 1) * 128],
                                     rhs=P["w1aug"][:], start=True, stop=True)
                rec_build(ph, 0, sc0)
                if g == S_H // 2 - 1:
                    allgather(0, 0)
            allgather(0, 1)

            # ---------------- E phase (layer L); chains T2 (L=0) or post (L=1)
            def e_prep(L, g):
                half = g // (NGRP // 2)
                lsc0 = (g * GRP) % S_H
                CH = NSEG * GRP          # 96 chunks
                gtA = gp.tile([128, 4, GRP, REC], F8, tag="gtA")
                gtB = gp.tile([128, 4, GRP, REC], F8, tag="gtB")
                srec = gp.tile([128, GRP, REC], F8, tag="srec")
                nc.sync.dma_start(srec[:], shp[L, half][:, lsc0:lsc0 + GRP, :])
                oh = go.tile([128, NSEG, GRP, 128], F8, tag="oh")
                nc.sync.dma_start(oh[:], din["ohs"].ap()[:, g * CH * 128:(g + 1) * CH * 128])
                ot = go.tile([128, NSEG, GRP, 128], F8, tag="ot")
                nc.sync.dma_start(ot[:], din["ots"].ap()[:, g * CH * 128:(g + 1) * CH * 128])
                C = GRP * 8
                for qh in range(NSEG):
                    hu = qh // 4
                    q = qh % 4
                    tgt = gtA if hu == 0 else gtB
                    nc.gpsimd.dma_gather(
                        tgt[:, q, :, :],
                        tab[L, hu].ap()[q * 2 * ROWS_H:(q + 1) * 2 * ROWS_H, :],
                        P["eidx"][:, (g * NSEG + qh) * C:(g * NSEG + qh + 1) * C],
                        GRP * 128, GRP * 128, REC,
                        single_packet=False, queue_num=qh % 4)
                # per-slot adst via oT matmuls
                pae = psB.tile([128, CH, 2], F32, tag="aux")
                for k in range(GRP):
                    for qh in range(NSEG):
                        cidx = qh * GRP + k
                        nc.tensor.matmul(pae[:, cidx, :],
                                         lhsT=ot[:, qh, k, :],
                                         rhs=aab[:, g * GRP + k, 2:4],
                                         start=True, stop=True)
                # logits -> weights; layout [A(48) | self(12) | B(48)]
                HC = CH // 2
                lg = sbE.tile([128, CH + GRP, 2], F32, tag="lg")
                gtfA = gtA[:].rearrange("p a b c -> p (a b) c")
                gtfB = gtB[:].rearrange("p a b c -> p (a b) c")
                nc.vector.tensor_tensor(lg[:, 0:HC, :],
                                        gtfA[:, :, 260:264].bitcast(BF),
                                        pae[:, 0:HC, :], ALU.add)
                nc.vector.tensor_tensor(lg[:, HC:HC + GRP, :],
                                        aab[:, g * GRP:(g + 1) * GRP, 0:2],
                                        aab[:, g * GRP:(g + 1) * GRP, 2:4], ALU.add)
                nc.vector.tensor_tensor(lg[:, HC + GRP:, :],
                                        gtfB[:, :, 260:264].bitcast(BF),
                                        pae[:, HC:CH, :], ALU.add)
                e1 = sbE.tile([128, CH + GRP, 2], BF, tag="e1")
                e2 = sbE.tile([128, CH + GRP, 2], BF, tag="e2")
                we = sbE.tile([128, CH + GRP, 2], BF, tag="we")
                NA = HC + GRP
                for lo, hi in ((0, NA), (NA, CH + GRP)):
                    nc.scalar.activation(e1[:, lo:hi, :], lg[:, lo:hi, :], AF.Exp)
                    nc.scalar.activation(e2[:, lo:hi, :], lg[:, lo:hi, :],
                                         AF.Exp, scale=NEG_SLOPE)
                    nc.vector.tensor_tensor(we[:, lo:hi, :], e1[:, lo:hi, :],
                                            e2[:, lo:hi, :], ALU.max)
                # weight records in place (per head block of 129 cols)
                for hd in range(2):
                    nc.vector.tensor_tensor(
                        gtfA[:, :, hd * 129:hd * 129 + 129],
                        gtfA[:, :, hd * 129:hd * 129 + 129],
                        we[:, 0:HC, hd:hd + 1].to_broadcast([128, HC, 129]),
                        ALU.mult)
                    nc.vector.tensor_tensor(
                        srec[:, :, hd * 129:hd * 129 + 129],
                        srec[:, :, hd * 129:hd * 129 + 129],
                        we[:, HC:HC + GRP, hd:hd + 1].to_broadcast([128, GRP, 129]),
                        ALU.mult)
                    nc.vector.tensor_tensor(
                        gtfB[:, :, hd * 129:hd * 129 + 129],
                        gtfB[:, :, hd * 129:hd * 129 + 129],
                        we[:, HC + GRP:, hd:hd + 1].to_broadcast([128, HC, 129]),
                        ALU.mult)
                return dict(gtA=gtA, gtB=gtB, srec=srec, oh=oh)

            def e_compute(L, g, T):
                CH = NSEG * GRP
                gtA, gtB, srec, oh = T["gtA"], T["gtB"], T["srec"], T["oh"]
                # aggregate + normalize per 2-sc pair
                for pr in range(GRP // 2):
                    po = psA.tile([128, 2, 512], F32, tag="po")
                    for ki in range(2):
                        k = pr * 2 + ki
                        nc.tensor.matmul(po[:, ki, 0:258], lhsT=P["identf8"][:],
                                         rhs=srec[:, k, 0:258], start=True, stop=False)
                        for qp in range(NSEG // 2):
                            gtX = gtA if qp < 2 else gtB
                            ql = (qp % 2) * 2
                            nc.tensor.matmul(
                                po[:, ki, 0:258],
                                lhsT=oh[:, 2 * qp:2 * qp + 2, k, :],
                                rhs=gtX[:, ql:ql + 2, k, 0:258],
                                start=False, stop=(qp == 3),
                                perf_mode=PM.DoubleRow)
                    po4 = po[:, :, 0:258].rearrange("p s (h c) -> p s h c", h=2)
                    den = sbE.tile([128, 2, 2, 1], F32, tag="den")
                    nc.vector.tensor_copy(den[:], po4[:, :, :, 128:129])
                    nc.vector.tensor_scalar_add(den[:], den[:], 1e-16)
                    nc.vector.reciprocal(den[:], den[:])
                    of = sbE.tile([128, 2, 2, 128], F32, tag="of")
                    nc.vector.tensor_tensor(
                        of[:], po4[:, :, :, 0:128],
                        den[:].to_broadcast([128, 2, 2, 128]),
                        ALU.mult)
                    bias = P["b1"] if L == 0 else P["b2"]
                    nc.vector.tensor_tensor(
                        of[:], of[:],
                        bias[:].unsqueeze(1).to_broadcast([128, 2, 2, 128]),
                        ALU.add)
                    o2 = sbE.tile([128, 2, 2, 128], BF, tag="o2")
                    nc.scalar.activation(o2[:], of[:], AF.Relu)
                    sc0 = g * GRP + pr * 2
                    if L == 0:
                        # chained T2: transpose o2, transform, write layer-2 recs
                        hT = sbT.tile([128, 2, 2, 128], BF, tag="hT")
                        for ki in range(2):
                            for ch in range(2):
                                pt = psC.tile([128, 128], BF, tag="pt")
                                nc.tensor.transpose(pt[:], o2[:, ki, ch, :],
                                                    P["identbf"][:])
                                nc.scalar.activation(hT[:, ki, ch, :], pt[:], AF.Copy)
                        ph = psA.tile([128, 2, 512], F32, tag="po")
                        for ki in range(2):
                            for ch in range(2):
                                nc.tensor.matmul(ph[:, ki, 0:260],
                                                 lhsT=hT[:, ki, ch, :],
                                                 rhs=w2s[:, ch, :],
                                                 start=(ch == 0), stop=(ch == 1))
                        rec_build(ph, 1, sc0)
                    else:
                        # chained post-MLP: y = sigmoid(o2 @ Wpf + bpf)
                        oT = sbT.tile([128, 2, 2, 128], BF, tag="hT")
                        for ki in range(2):
                            for ch in range(2):
                                pt = psC.tile([128, 128], BF, tag="pt")
                                nc.tensor.transpose(pt[:], o2[:, ki, ch, :],
                                                    P["identbf"][:])
                                nc.scalar.activation(oT[:, ki, ch, :], pt[:], AF.Copy)
                        yp = psC.tile([128, 2, 2], F32, tag="pt")
                        for ki in range(2):
                            for ch in range(2):
                                nc.tensor.matmul(yp[:, ki, :],
                                                 lhsT=oT[:, ki, ch, :],
                                                 rhs=wpf[:, ch, :],
                                                 start=(ch == 0), stop=(ch == 1))
                        yv = sbE.tile([128, 2, 2], F32, tag="yv")
                        nc.vector.tensor_tensor(
                            yv[:], yp[:],
                            P["bpf"][:].unsqueeze(1).to_broadcast([128, 2, 2]),
                            ALU.add)
                        sg = sbE.tile([128, 2, 2], F32, tag="sg")
                        nc.scalar.activation(sg[:], yv[:], AF.Exp, scale=-1.0)
                        nc.vector.tensor_scalar_add(sg[:], sg[:], 1.0)
                        nc.vector.reciprocal(sg[:], sg[:])
                        ypd = y_d.ap().rearrange("(s p) c -> p s c", p=128)
                        nc.scalar.dma_start(ypd[:, sc0:sc0 + 2, :], sg[:])

            for L01 in range(2):
                prev = None
                for g in range(NGRP + 1):
                    cur = e_prep(L01, g) if g < NGRP else None
                    if prev is not None:
                        e_compute(L01, g - 1, prev)
                        if L01 == 0 and g - 1 == NGRP // 2 - 1:
                            allgather(1, 0)
                        if L01 == 0 and g - 1 == NGRP - 1:
                            allgather(1, 1)
                    prev = cur
    nc.compile()
    return nc


_NC_CACHE = None


def kernel(**inputs):
    global _NC_CACHE
    from concourse.bass_utils import run_bass_kernel_spmd

    plan = build_plan(inputs["edge_index"])
    cores, shared = make_core_inputs(plan, inputs)

    if _NC_CACHE is None:
        _NC_CACHE = build_nc()
    nc = _NC_CACHE

    in_maps = []
    for c in range(8):
        m = dict(shared)
        m.update(cores[c])
        in_maps.append({k: np.ascontiguousarray(v) for k, v in m.items()})

    res = run_bass_kernel_spmd(nc, in_maps, core_ids=list(range(8)))

    owner, sc_of, row_of = plan["owner"], plan["sc_of"], plan["row_of"]
    y = np.zeros((N, 2), dtype=np.float32)
    for c in range(8):
        yc = res.results[c]["y"]
        nodes = np.where(owner == c)[0]
        y[nodes] = yc[sc_of[nodes] * 128 + row_of[nodes]]
    return y


# revision 13
# speedup vs baseline: 1.0565x; 1.0488x over previous
"""Trainium2 Bass kernel for 2-layer GAT (nn_GAT_59133109732231). v3: fp8.

Self-contained: kernel(**inputs) -> np.ndarray [100000, 2] float32.

Distribution (8 NeuronCores, SPMD), node-parallel:
  - nodes permuted: core c owns 120 superchunks (sc) x 128 rows (row 127 =
    trash row). sc 0..59 = half A, 60..119 = half B.
  - per sc, in-edges packed into 8 segments of 128 slots keyed by the
    SOURCE's (core-pair q, half h) octant -> one gather chunk each.
  - per layer: transform nodes with augmented weights [W | W@a_src | W@a_dst]
    (one matmul -> h + both attention dots), write 512B fp8 records
    [h0 |1| h1 |1| pad | asrc(bf16 pair) | pad]; TWO AllGathers (half A
    during half-B transform) -> record tables; per 12-sc group dma_gather
    512B records by int16 row id; per-edge weights w=exp(lrelu(asr+adst))
    via exp-max identity (scalar engine runs only Exp); weight the gathered
    records in place; aggregate + softmax denominator with fp8 DoubleRow
    one-hot matmuls in PSUM (host-streamed one-hot tables); adst per edge
    gathered via tiny oT matmuls. Layer-2 transform / post-MLP (Wp1@Wp2
    folded) chained in SBUF per 2-sc pair.
"""
import os
import sys

import numpy as np
import ml_dtypes

for _p in ("/opt/trn_rl_repo", "/root/.axon_site/_ro/trn_rl_repo"):
    if os.path.isdir(_p) and _p not in sys.path:
        sys.path.append(_p)

N = 100000
NCORES = 8
S_SC = 120            # superchunks per core
S_H = 60              # sc per half
ROWS_CORE = S_SC * 128
ROWS_H = S_H * 128    # 7680
REC = 512             # fp8 cols per record (512 B)
GRP = 12              # sc per E-group
NGRP = S_SC // GRP    # 10
NEG_SLOPE = 0.2
NSEG = 8              # segments (octants) per sc

bf16 = ml_dtypes.bfloat16
fp8 = ml_dtypes.float8_e4m3


# ----------------------------------------------------------------- host prep
def build_plan(edge_index):
    edge_index = np.asarray(edge_index)
    src = edge_index[0].astype(np.int64)
    dst = edge_index[1].astype(np.int64)

    deg = np.bincount(dst, minlength=N)
    order = np.argsort(-deg, kind="stable")
    owner = np.empty(N, dtype=np.int32)
    snake = np.tile(np.concatenate([np.arange(8), np.arange(7, -1, -1)]),
                    N // 16 + 1)[:N]
    owner[order] = snake.astype(np.int32)

    # half assignment: alternate within each core by degree rank
    half = np.empty(N, dtype=np.int32)
    for c in range(NCORES):
        nodes = order[owner[order] == c]
        half[nodes] = np.arange(len(nodes)) % 2

    # octant of an edge = (src core-pair, src half)
    e_oct = (owner[src] // 2) * 2 + half[src]
    qd = np.zeros((N, NSEG), dtype=np.int32)
    np.add.at(qd, (dst, e_oct), 1)

    sc_of = np.empty(N, dtype=np.int32)
    row_of = np.empty(N, dtype=np.int32)
    for c in range(NCORES):
        for h in range(2):
            nodes = np.where((owner == c) & (half == h))[0]
            nodes = nodes[np.argsort(-deg[nodes], kind="stable")]
            loads = np.zeros((S_H, NSEG), dtype=np.int32)
            counts = np.zeros(S_H, dtype=np.int32)
            tot = np.zeros(S_H, dtype=np.int32)
            big = 1.0e9
            for n in nodes:
                after = loads + qd[n][None, :]
                ok = (after <= 128).all(axis=1) & (counts < 127)
                key = after.max(axis=1).astype(np.float64) + tot * 1e-6 + (~ok) * big
                k = int(np.argmin(key))
                assert ok[k], "packing failed"
                sc_of[n] = h * S_H + k
                row_of[n] = counts[k]
                counts[k] += 1
                loads[k] += qd[n]
                tot[k] += deg[n]
    # eidx value: row within the (q, h) table region
    rowq_of = ((owner % 2) * ROWS_H + (sc_of % S_H) * 128 + row_of).astype(np.int32)

    e_core = owner[dst]
    e_sc = sc_of[dst]
    e_rowq = rowq_of[src]
    e_dloc = row_of[dst]

    plans = []
    for c in range(NCORES):
        eidx = np.zeros((S_SC, NSEG, 128), dtype=np.int16)
        dloc = np.full((S_SC, NSEG, 128), 127, dtype=np.int32)
        m = e_core == c
        sc_c, o_c, rq_c, dl_c = e_sc[m], e_oct[m], e_rowq[m], e_dloc[m]
        o = np.lexsort((o_c, sc_c))
        sc_c, o_c, rq_c, dl_c = sc_c[o], o_c[o], rq_c[o], dl_c[o]
        key = sc_c * NSEG + o_c
        pos = np.arange(len(key)) - np.searchsorted(key, key, side="left")
        assert pos.max() < 128
        eidx[sc_c, o_c, pos] = rq_c.astype(np.int16)
        dloc[sc_c, o_c, pos] = dl_c
        plans.append(dict(eidx=eidx, dloc=dloc))
    return dict(owner=owner, sc_of=sc_of, row_of=row_of, plans=plans)


def make_core_inputs(plan, inputs):
    x = np.asarray(inputs["x"], dtype=np.float32)

    def amat(a):
        a = np.asarray(a, dtype=np.float32)
        m = np.zeros((256, 2), dtype=np.float32)
        m[0:128, 0] = a[0]
        m[128:256, 1] = a[1]
        return m

    W1 = np.asarray(inputs["W1"], dtype=np.float32)
    W2 = np.asarray(inputs["W2"], dtype=np.float32)
    W1aug = np.concatenate(
        [W1, W1 @ amat(inputs["a_src1"]), W1 @ amat(inputs["a_dst1"])], axis=1)
    W2aug = np.concatenate(
        [W2, W2 @ amat(inputs["a_src2"]), W2 @ amat(inputs["a_dst2"])], axis=1)
    Wp1 = np.asarray(inputs["Wp1"], dtype=np.float32)
    Wp2 = np.asarray(inputs["Wp2"], dtype=np.float32)
    Wpf = Wp1 @ Wp2                                   # [256, 2]
    bpf = np.asarray(inputs["bp1"], np.float32) @ Wp2 + np.asarray(
        inputs["bp2"], np.float32)                    # [2]

    rep = lambda v, d: np.broadcast_to(
        np.asarray(v, dtype=np.float32)[None, :], (128, d)).copy()
    shared = dict(
        w1aug=W1aug.astype(bf16),
        w2aug=W2aug.astype(bf16),
        wpf=Wpf.astype(bf16),
        b1=rep(inputs["b1"], 256).reshape(128, 2, 128).copy(),
        b2=rep(inputs["b2"], 256).reshape(128, 2, 128).copy(),
        bpf=rep(bpf, 2),
        identf8=np.eye(128, dtype=np.float32).astype(fp8),
        identbf=np.eye(128, dtype=np.float32).astype(bf16),
    )

    owner, sc_of, row_of = plan["owner"], plan["sc_of"], plan["row_of"]
    ar = np.arange(128)
    cores = []
    for c in range(NCORES):
        xpT = np.zeros((128, ROWS_CORE), dtype=np.float32)
        nodes = np.where(owner == c)[0]
        xpT[:, sc_of[nodes] * 128 + row_of[nodes]] = x[nodes].T
        p = plan["plans"][c]
        # reorder octants hu-major: qh = hu*4 + q  (oct was q*2+hu)
        perm = [0, 2, 4, 6, 1, 3, 5, 7]
        dlocp = p["dloc"][:, perm, :]
        eidxp = p["eidx"][:, perm, :]
        # one-hot streams: chunk index c = qh*GRP + k within group g
        A = (dlocp[:, :, :, None] == ar).astype(np.uint8)  # [sc, qh, s, m]
        A5 = A.reshape(NGRP, GRP, NSEG, 128, 128)
        ohs = np.ascontiguousarray(
            A5.transpose(3, 0, 2, 1, 4).reshape(128, NGRP * NSEG * GRP * 128))
        ots = np.ascontiguousarray(
            A5.transpose(4, 0, 2, 1, 3).reshape(128, NGRP * NSEG * GRP * 128))
        # eidx wrap: per (g, qh) gather of GRP*128 idxs
        ei = eidxp.reshape(NGRP, GRP, NSEG, 128)
        eiw = np.zeros((128, NGRP * NSEG * GRP * 8), dtype=np.int16)
        C = GRP * 8  # cols per gather = GRP*128/16
        for g in range(NGRP):
            for qh in range(NSEG):
                lst = ei[g, :, qh, :].reshape(-1)          # [GRP*128]
                w = lst.reshape(GRP * 8, 16).T             # [16, GRP*8]
                eiw[:, (g * NSEG + qh) * C:(g * NSEG + qh + 1) * C] = np.tile(w, (8, 1))
        cores.append(dict(xpT=xpT.astype(bf16),
                          eidx=eiw,
                          ohs=ohs.astype(fp8),
                          ots=ots.astype(fp8)))
    return cores, shared


# -------------------------------------------------------------- bass program
def build_nc():
    import concourse.bass as bass
    import concourse.bacc as bacc
    import concourse.mybir as mybir
    import concourse.tile as tile

    F32, BF, F8, I16 = (mybir.dt.float32, mybir.dt.bfloat16,
                        mybir.dt.float8e4, mybir.dt.int16)
    AF = mybir.ActivationFunctionType
    ALU = mybir.AluOpType
    PM = mybir.MatmulPerfMode

    nc = bacc.Bacc("TRN2", target_bir_lowering=False, debug=False,
                   num_devices=8, num_swdge_queues=4)

    din = {}
    for name, shape, dt in [
        ("xpT", [128, ROWS_CORE], BF),
        ("eidx", [128, NGRP * NSEG * GRP * 8], I16),
        ("ohs", [128, NGRP * NSEG * GRP * 128], F8),
        ("ots", [128, NGRP * NSEG * GRP * 128], F8),
        ("w1aug", [128, 260], BF),
        ("w2aug", [256, 260], BF),
        ("wpf", [256, 2], BF),
        ("b1", [128, 2, 128], F32), ("b2", [128, 2, 128], F32),
        ("bpf", [128, 2], F32),
        ("identf8", [128, 128], F8),
        ("identbf", [128, 128], BF),
    ]:
        din[name] = nc.dram_tensor(name, shape, dt, kind="ExternalInput")
    y_d = nc.dram_tensor("y", [ROWS_CORE, 2], F32, kind="ExternalOutput")
    sh = {}
    tab = {}
    for L in range(2):
        for h in range(2):
            sh[L, h] = nc.dram_tensor(f"sh{L}{h}", [ROWS_H, REC], F8,
                                      kind="Internal")
            tab[L, h] = nc.dram_tensor(f"tab{L}{h}", [8 * ROWS_H, REC], F8,
                                       kind="Internal", addr_space="Shared")

    with tile.TileContext(nc) as tc:
        import contextlib
        ctx = contextlib.ExitStack()
        with ctx:
            pp = ctx.enter_context(tc.tile_pool(name="pp", bufs=1))
            sbT = ctx.enter_context(tc.tile_pool(name="sbT", bufs=2))
            gp = ctx.enter_context(tc.tile_pool(name="gp", bufs=2))
            go = ctx.enter_context(tc.tile_pool(name="go", bufs=2))
            sbE = ctx.enter_context(tc.tile_pool(name="sbE", bufs=2))
            psA = ctx.enter_context(tc.tile_pool(name="psA", bufs=2, space="PSUM"))
            psB = ctx.enter_context(tc.tile_pool(name="psB", bufs=2, space="PSUM"))
            psC = ctx.enter_context(tc.tile_pool(name="psC", bufs=2, space="PSUM"))

            # persistent SBUF
            P = {}
            for name in ("eidx", "w1aug", "b1", "b2", "bpf", "identf8", "identbf"):
                t = pp.tile(list(din[name].shape), din[name].dtype, tag=f"p_{name}")
                nc.sync.dma_start(t[:], din[name].ap())
                P[name] = t
            w2s = pp.tile([128, 2, 260], BF, tag="p_w2")
            nc.sync.dma_start(w2s[:, 0, :], din["w2aug"].ap()[0:128, :])
            nc.sync.dma_start(w2s[:, 1, :], din["w2aug"].ap()[128:256, :])
            wpf = pp.tile([128, 2, 2], BF, tag="p_wpf")
            nc.sync.dma_start(wpf[:, 0, :], din["wpf"].ap()[0:128, :])
            nc.sync.dma_start(wpf[:, 1, :], din["wpf"].ap()[128:256, :])
            aab = pp.tile([128, S_SC, 4], BF, tag="p_aab")

            shp = {k: v.ap().rearrange("(s p) c -> p s c", p=128)
                   for k, v in sh.items()}

            def rec_build(ph, L, sc0):
                """ph [128,2,512] f32 psum (cols 0:260 = h|asrc|adst) ->
                fp8 records + aab; DMA to shard."""
                h = sc0 // S_H
                lsc = sc0 % S_H
                rec = sbT.tile([128, 2, REC], F8, tag="rec")
                nc.scalar.activation(rec[:, :, 0:128], ph[:, :, 0:128], AF.Copy)
                nc.vector.tensor_copy(rec[:, :, 129:257], ph[:, :, 128:256])
                nc.vector.memset(rec[:, :, 128:129], 1.0)
                nc.vector.memset(rec[:, :, 257:258], 1.0)
                nc.scalar.activation(rec[:, :, 260:264].bitcast(BF),
                                     ph[:, :, 256:258], AF.Copy)
                nc.vector.tensor_copy(aab[:, sc0:sc0 + 2, :], ph[:, :, 256:260])
                nc.sync.dma_start(shp[L, h][:, lsc:lsc + 2, :], rec[:])

            def allgather(L, h):
                import concourse.mybir as mybir
                nc.gpsimd.collective_compute(
                    "AllGather", mybir.AluOpType.bypass,
                    replica_groups=[list(range(8))],
                    ins=[sh[L, h].ap()], outs=[tab[L, h].ap()])

            # ---------------- phase T1
            for g in range(S_H):   # 60 groups of 2 sc
                sc0 = g * 2
                xt = sbT.tile([128, 256], BF, tag="xt1")
                nc.sync.dma_start(xt[:], din["xpT"].ap()[:, sc0 * 128:(sc0 + 2) * 128])
                ph = psA.tile([128, 2, 512], F32, tag="po")
                for ki in range(2):
                    nc.tensor.matmul(ph[:, ki, 0:260], lhsT=xt[:, ki * 128:(ki + 1) * 128],
                                     rhs=P["w1aug"][:], start=True, stop=True)
                rec_build(ph, 0, sc0)
                if g == S_H // 2 - 1:
                    allgather(0, 0)
            allgather(0, 1)

            # ---------------- E phase (layer L); chains T2 (L=0) or post (L=1)
            def e_prep(L, g):
                half = g // (NGRP // 2)
                lsc0 = (g * GRP) % S_H
                CH = NSEG * GRP          # 96 chunks
                gtA = gp.tile([128, 4, GRP, REC], F8, tag="gtA")
                gtB = gp.tile([128, 4, GRP, REC], F8, tag="gtB")
                srec = gp.tile([128, GRP, REC], F8, tag="srec")
                nc.sync.dma_start(srec[:], shp[L, half][:, lsc0:lsc0 + GRP, :])
                oh = go.tile([128, NSEG, GRP, 128], F8, tag="oh")
                nc.sync.dma_start(oh[:], din["ohs"].ap()[:, g * CH * 128:(g + 1) * CH * 128])
                ot = go.tile([128, NSEG, GRP, 128], F8, tag="ot")
                nc.sync.dma_start(ot[:], din["ots"].ap()[:, g * CH * 128:(g + 1) * CH * 128])
                C = GRP * 8
                for qh in range(NSEG):
                    hu = qh // 4
                    q = qh % 4
                    tgt = gtA if hu == 0 else gtB
                    nc.gpsimd.dma_gather(
                        tgt[:, q, :, :],
                        tab[L, hu].ap()[q * 2 * ROWS_H:(q + 1) * 2 * ROWS_H, :],
                        P["eidx"][:, (g * NSEG + qh) * C:(g * NSEG + qh + 1) * C],
                        GRP * 128, GRP * 128, REC,
                        single_packet=False, queue_num=qh % 4)
                # per-slot adst via oT matmuls
                pae = psB.tile([128, CH, 2], F32, tag="aux")
                for k in range(GRP):
                    for qh in range(NSEG):
                        cidx = qh * GRP + k
                        nc.tensor.matmul(pae[:, cidx, :],
                                         lhsT=ot[:, qh, k, :],
                                         rhs=aab[:, g * GRP + k, 2:4],
                                         start=True, stop=True)
                # logits -> weights; layout [A(48) | self(12) | B(48)]
                HC = CH // 2
                lg = sbE.tile([128, CH + GRP, 2], F32, tag="lg")
                gtfA = gtA[:].rearrange("p a b c -> p (a b) c")
                gtfB = gtB[:].rearrange("p a b c -> p (a b) c")
                nc.vector.tensor_tensor(lg[:, 0:HC, :],
                                        gtfA[:, :, 260:264].bitcast(BF),
                                        pae[:, 0:HC, :], ALU.add)
                nc.vector.tensor_tensor(lg[:, HC:HC + GRP, :],
                                        aab[:, g * GRP:(g + 1) * GRP, 0:2],
                                        aab[:, g * GRP:(g + 1) * GRP, 2:4], ALU.add)
                nc.vector.tensor_tensor(lg[:, HC + GRP:, :],
                                        gtfB[:, :, 260:264].bitcast(BF),
                                        pae[:, HC:CH, :], ALU.add)
                e1 = sbE.tile([128, CH + GRP, 2], BF, tag="e1")
                e2 = sbE.tile([128, CH + GRP, 2], BF, tag="e2")
                we = sbE.tile([128, CH + GRP, 2], BF, tag="we")
                NA = HC + GRP
                for lo, hi in ((0, NA), (NA, CH + GRP)):
                    nc.scalar.activation(e1[:, lo:hi, :], lg[:, lo:hi, :], AF.Exp)
                    nc.scalar.activation(e2[:, lo:hi, :], lg[:, lo:hi, :],
                                         AF.Exp, scale=NEG_SLOPE)
                    nc.vector.tensor_tensor(we[:, lo:hi, :], e1[:, lo:hi, :],
                                            e2[:, lo:hi, :], ALU.max)
                # weight records in place (per head block of 129 cols)
                for hd in range(2):
                    nc.vector.tensor_tensor(
                        gtfA[:, :, hd * 129:hd * 129 + 129],
                        gtfA[:, :, hd * 129:hd * 129 + 129],
                        we[:, 0:HC, hd:hd + 1].to_broadcast([128, HC, 129]),
                        ALU.mult)
                    nc.vector.tensor_tensor(
                        srec[:, :, hd * 129:hd * 129 + 129],
                        srec[:, :, hd * 129:hd * 129 + 129],
                        we[:, HC:HC + GRP, hd:hd + 1].to_broadcast([128, GRP, 129]),
                        ALU.mult)
                    nc.vector.tensor_tensor(
                        gtfB[:, :, hd * 129:hd * 129 + 129],
                        gtfB[:, :, hd * 129:hd * 129 + 129],
                        we[:, HC + GRP:, hd:hd + 1].to_broadcast([128, HC, 129]),
                        ALU.mult)
                return dict(gtA=gtA, gtB=gtB, srec=srec, oh=oh)

            def e_compute(L, g, T):
                CH = NSEG * GRP
                gtA, gtB, srec, oh = T["gtA"], T["gtB"], T["srec"], T["oh"]
                # aggregate + normalize per 2-sc pair
                for pr in range(GRP // 2):
                    po = psA.tile([128, 2, 512], F32, tag="po")
                    for ki in range(2):
                        k = pr * 2 + ki
                        nc.tensor.matmul(po[:, ki, 0:258], lhsT=P["identf8"][:],
                                         rhs=srec[:, k, 0:258], start=True, stop=False)
                        for qp in range(NSEG // 2):
                            gtX = gtA if qp < 2 else gtB
                            ql = (qp % 2) * 2
                            nc.tensor.matmul(
                                po[:, ki, 0:258],
                                lhsT=oh[:, 2 * qp:2 * qp + 2, k, :],
                                rhs=gtX[:, ql:ql + 2, k, 0:258],
                                start=False, stop=(qp == 3),
                                perf_mode=PM.DoubleRow)
                    po4 = po[:, :, 0:258].rearrange("p s (h c) -> p s h c", h=2)
                    den = sbE.tile([128, 2, 2, 1], F32, tag="den")
                    nc.scalar.activation(den[:], po4[:, :, :, 128:129], AF.Copy,
                                         bias=1e-16)
                    nc.vector.reciprocal(den[:], den[:])
                    of = sbE.tile([128, 2, 2, 128], F32, tag="of")
                    nc.vector.tensor_tensor(
                        of[:], po4[:, :, :, 0:128],
                        den[:].to_broadcast([128, 2, 2, 128]),
                        ALU.mult)
                    bias = P["b1"] if L == 0 else P["b2"]
                    nc.vector.tensor_tensor(
                        of[:], of[:],
                        bias[:].unsqueeze(1).to_broadcast([128, 2, 2, 128]),
                        ALU.add)
                    o2 = sbE.tile([128, 2, 2, 128], BF, tag="o2")
                    nc.scalar.activation(o2[:], of[:], AF.Relu)
                    sc0 = g * GRP + pr * 2
                    if L == 0:
                        # chained T2: transpose o2, transform, write layer-2 recs
                        hT = sbT.tile([128, 2, 2, 128], BF, tag="hT")
                        for ki in range(2):
                            for ch in range(2):
                                pt = psC.tile([128, 128], BF, tag="pt")
                                nc.tensor.transpose(pt[:], o2[:, ki, ch, :],
                                                    P["identbf"][:])
                                nc.scalar.activation(hT[:, ki, ch, :], pt[:], AF.Copy)
                        ph = psA.tile([128, 2, 512], F32, tag="po")
                        for ki in range(2):
                            for ch in range(2):
                                nc.tensor.matmul(ph[:, ki, 0:260],
                                                 lhsT=hT[:, ki, ch, :],
                                                 rhs=w2s[:, ch, :],
                                                 start=(ch == 0), stop=(ch == 1))
                        rec_build(ph, 1, sc0)
                    else:
                        # chained post-MLP: y = sigmoid(o2 @ Wpf + bpf)
                        oT = sbT.tile([128, 2, 2, 128], BF, tag="hT")
                        for ki in range(2):
                            for ch in range(2):
                                pt = psC.tile([128, 128], BF, tag="pt")
                                nc.tensor.transpose(pt[:], o2[:, ki, ch, :],
                                                    P["identbf"][:])
                                nc.scalar.activation(oT[:, ki, ch, :], pt[:], AF.Copy)
                        yp = psC.tile([128, 2, 2], F32, tag="pt")
                        for ki in range(2):
                            for ch in range(2):
                                nc.tensor.matmul(yp[:, ki, :],
                                                 lhsT=oT[:, ki, ch, :],
                                                 rhs=wpf[:, ch, :],
                                                 start=(ch == 0), stop=(ch == 1))
                        yv = sbE.tile([128, 2, 2], F32, tag="yv")
                        nc.vector.tensor_tensor(
                            yv[:], yp[:],
                            P["bpf"][:].unsqueeze(1).to_broadcast([128, 2, 2]),
                            ALU.add)
                        sg = sbE.tile([128, 2, 2], F32, tag="sg")
                        nc.scalar.activation(sg[:], yv[:], AF.Exp, scale=-1.0)
                        nc.vector.tensor_scalar_add(sg[:], sg[:], 1.0)
                        nc.vector.reciprocal(sg[:], sg[:])
                        ypd = y_d.ap().rearrange("(s p) c -> p s c", p=128)
                        nc.scalar.dma_start(ypd[:, sc0:sc0 + 2, :], sg[:])

            for L01 in range(2):
                prev = None
                for g in range(NGRP + 1):
                    cur = e_prep(L01, g) if g < NGRP else None
                    if prev is not None:
                        e_compute(L01, g - 1, prev)
                        if L01 == 0 and g - 1 == NGRP // 2 - 1:
                            allgather(1, 0)
                        if L01 == 0 and g - 1 == NGRP - 1:
                            allgather(1, 1)
                    prev = cur
    nc.compile()
    return nc


_NC_CACHE = None


def kernel(**inputs):
    global _NC_CACHE
    from concourse.bass_utils import run_bass_kernel_spmd

    plan = build_plan(inputs["edge_index"])
    cores, shared = make_core_inputs(plan, inputs)

    if _NC_CACHE is None:
        _NC_CACHE = build_nc()
    nc = _NC_CACHE

    in_maps = []
    for c in range(8):
        m = dict(shared)
        m.update(cores[c])
        in_maps.append({k: np.ascontiguousarray(v) for k, v in m.items()})

    res = run_bass_kernel_spmd(nc, in_maps, core_ids=list(range(8)))

    owner, sc_of, row_of = plan["owner"], plan["sc_of"], plan["row_of"]
    y = np.zeros((N, 2), dtype=np.float32)
    for c in range(8):
        yc = res.results[c]["y"]
        nodes = np.where(owner == c)[0]
        y[nodes] = yc[sc_of[nodes] * 128 + row_of[nodes]]
    return y
